# revision 21
# baseline (speedup 1.0000x reference)
"""DecayAttention Trainium2 kernel — 8-core SPMD.

Problem: B=2, L=2048, D=1024, H=16 heads (Hd=64).
  out = (softmax(Q K^T/sqrt(Hd) - rate_h*log1p(|i-j|) + causal) V) @ Wo.T + bo

Sharding: core c handles batch b = c//4 and heads h in [4*(c%4), 4*(c%4)+4).
Q/K/V projections column-sharded, Wo row-sharded; the 4 cores of each batch
return partial outputs that the host sums (plus Wo@bv + bo, both of which are
q-independent constants because softmax rows sum to 1).

Device-side layout tricks:
  - x is shipped pre-transposed (xT [D, L]) so every matmul contraction dim
    sits on partitions; no on-device transposes at all.
  - Q^T/K^T [64, L] per head come straight out of the projection matmuls.
  - scores are computed transposed (S^T[k, q] = K Q^T) so softmax's k-reduction
    becomes a matmul contraction: V is augmented with a ones column and
    P^T = exp(S^T) * expA gives numerator and denominator in one PV matmul.
  - decay bias + causal mask collapse into one Toeplitz factor
    expA[k, q] = (1+|q-k|)^(-rate) * [k <= q], materialized per tile by a
    single DMA from a 4095-float vector with a diagonal access pattern
    [[1, 128], [-1, 512]].
  - matmul operands use float32r (TF32, full PE rate at N>=256).
"""
import math

import numpy as np

import concourse.bass as bass
import concourse.mybir as mybir
import concourse.tile as tile
from concourse import bass_utils

f32 = mybir.dt.float32
f32r = mybir.dt.float32r
Exp = mybir.ActivationFunctionType.Exp

B, L, D, H = 2, 2048, 1024, 16
Hd = D // H                      # 64
N_CORES = 8
CPB = N_CORES // B               # 4 cores per batch element
HPC = H // CPB                   # 4 heads per core
DHC = HPC * Hd                   # 256 head-dims per core
NQ = L // 512                    # 4 q-chunks of 512
NLT = L // 128                   # 16 l/k tiles of 128
NE = D // 128                    # 8 contraction tiles for projections
GLEN = 2 * L - 1                 # 4095
GOFF = L - 1                     # 2047


def _split_multi_waits(nc):
    """This container's walrus accepts at most one sync-wait per engine
    instruction; hoist extras onto single-wait NOPs placed just before."""
    for fn in nc.m.functions:
        for blk in fn.blocks:
            out, changed = [], False
            for inst in blk.instructions:
                si = inst.sync_info
                waits = list(si.on_wait) if si is not None and si.on_wait else []
                if len(waits) > 1:
                    changed = True
                    for w in waits[:-1]:
                        nop = mybir.InstNoOp(
                            name=nc.get_next_instruction_name(), ins=[], outs=[])
                        nop.engine = inst.engine
                        nop.sync_info = mybir.SyncInfo(on_wait=[w], on_update=[])
                        out.append(nop)
                    inst.sync_info = mybir.SyncInfo(
                        on_wait=[waits[-1]], on_update=list(si.on_update or []))
                out.append(inst)
            if changed:
                blk.instructions = out


def build_nc(n_g: int, phases=("A", "B", "WO"), repeat=1, internal_io=False):
    """Build the per-core Bass program. n_g = 1 (all heads share one decay
    rate, the setup_inputs case) or HPC (per-head expA vectors)."""
    nc = bass.Bass("TRN2", target_bir_lowering=False, debug=False)

    big = "Internal" if internal_io else "ExternalInput"
    xT = nc.dram_tensor("xT", [D, L], f32r, kind=big).ap()
    wqT = nc.dram_tensor("wqT", [D, DHC], f32r, kind=big).ap()
    wkT = nc.dram_tensor("wkT", [D, DHC], f32r, kind=big).ap()
    wvT = nc.dram_tensor("wvT", [D, DHC], f32r, kind=big).ap()
    woT = nc.dram_tensor("woT", [DHC, D], f32r, kind=big).ap()
    bq = nc.dram_tensor("bq", [DHC, 1], f32, kind="ExternalInput").ap()
    bk = nc.dram_tensor("bk", [DHC, 1], f32, kind="ExternalInput").ap()
    g = nc.dram_tensor("g", [n_g, GLEN], f32, kind="ExternalInput")
    pmask = nc.dram_tensor("pmask", [L, 1], f32, kind="ExternalInput").ap()
    out = nc.dram_tensor(
        "out", [L, D], f32,
        kind="Internal" if internal_io else "ExternalOutput").ap()
    tok = (nc.dram_tensor("tok", [128, 1], f32, kind="ExternalOutput").ap()
           if internal_io else None)

    with tile.TileContext(nc) as tc:
      for _rep in range(repeat):
        with tc.tile_pool(name="cons", bufs=1) as cons:
            # persistent SBUF residents (pair layout: pair p = heads 2p, 2p+1)
            NP = HPC // 2
            qt_p = [cons.tile([128, L], f32r, name=f"qt{p}") for p in range(NP)]
            kt_p = [cons.tile([128, L], f32r, name=f"kt{p}") for p in range(NP)]
            vaug = [cons.tile([128, 65 * HPC], f32r, name=f"vaug{t}")
                    for t in range(NLT)]
            wo_p = [cons.tile([128, D], f32r, name=f"wo{p}") for p in range(NP)]
            bq_p = [cons.tile([128, 1], f32, name=f"bq{p}") for p in range(NP)]
            bk_p = [cons.tile([128, 1], f32, name=f"bk{p}") for p in range(NP)]
            ones64 = cons.tile([1, Hd], f32r)
            ones_st = cons.tile([128, HPC], f32)

            ones_st64 = cons.tile([1, Hd], f32)
            nc.vector.memset(ones_st[:, :], 1.0)
            nc.vector.memset(ones_st64[:, :], 1.0)
            nc.vector.tensor_copy(ones64[:, :], ones_st64[:, :])
            for p in range(NP):
                nc.sync.dma_start(wo_p[p][:, :], woT[p * 128:(p + 1) * 128, :])
                nc.sync.dma_start(bq_p[p][:, :], bq[p * 128:(p + 1) * 128, :])
                nc.sync.dma_start(bk_p[p][:, :], bk[p * 128:(p + 1) * 128, :])

            # ---- Phase A + B pools (single PSUM pool layout: tag "s"
            # slots are [128,1024]=2 banks x2 bufs; phase-A tiles borrow them)
            with tc.tile_pool(name="eap", bufs=5) as eap, \
                 tc.tile_pool(name="wrk", bufs=3) as wrk, \
                 tc.tile_pool(name="otp", bufs=2) as otp, \
                 tc.tile_pool(name="psS", bufs=2, space="PSUM") as psS, \
                 tc.tile_pool(name="psV", bufs=HPC, space="PSUM") as psV, \
                 tc.tile_pool(name="xw", bufs=1) as xw:
                psA = psS
                xt_t = [xw.tile([128, L], f32r, name=f"x{e}") for e in range(NE)]
                wq_t = [xw.tile([128, DHC], f32r, name=f"wq{e}") for e in range(NE)]
                wk_t = [xw.tile([128, DHC], f32r, name=f"wk{e}") for e in range(NE)]
                wv_t = [xw.tile([128, DHC], f32r, name=f"wv{e}") for e in range(NE)]
                pm_t = [xw.tile([128, 1], f32, name=f"pm{t}") for t in range(NLT)]
                for e in range(NE):
                    nc.sync.dma_start(xt_t[e][:, :], xT[e * 128:(e + 1) * 128, :])
                    nc.sync.dma_start(wq_t[e][:, :], wqT[e * 128:(e + 1) * 128, :])
                    nc.sync.dma_start(wk_t[e][:, :], wkT[e * 128:(e + 1) * 128, :])
                    nc.sync.dma_start(wv_t[e][:, :], wvT[e * 128:(e + 1) * 128, :])
                for t in range(NLT):
                    nc.sync.dma_start(pm_t[t][:, :], pmask[t * 128:(t + 1) * 128, :])

                # Projections grouped by q-chunk: V l-tiles 4qc..4qc+3,
                # then K and Q for that chunk — attention(qc) unblocks early.
                def v_tile(t):
                    pv = psA.tile([128, DHC], f32, name="pv", tag="s")
                    for e in range(NE):
                        nc.tensor.matmul(
                            pv[:, :], xt_t[e][:, t * 128:(t + 1) * 128],
                            wv_t[e][:, :],
                            start=(e == 0), stop=(e == NE - 1))
                    dst = bass.AP(vaug[t].tensor, 0,
                                  [[65 * HPC, 128], [65, HPC], [1, Hd]])
                    src_ = bass.AP(pv.tensor, 0,
                                   [[DHC, 128], [Hd, HPC], [1, Hd]])
                    nc.vector.tensor_scalar_mul(dst, src_, pm_t[t][:, :])
                    ones_dst = bass.AP(vaug[t].tensor, Hd,
                                       [[65 * HPC, 128], [65, HPC]])
                    nc.vector.tensor_scalar_mul(ones_dst, ones_st[:, :],
                                                pm_t[t][:, :])

                for q in range(NQ):
                    for t in range(4 * q, 4 * q + 4):
                        v_tile(t)
                    for p in range(NP):
                        ps_ = p * 128
                        pk = psA.tile([128, 512], f32, name="pk", tag="s")
                        for e in range(NE):
                            nc.tensor.matmul(
                                pk[:, :], wk_t[e][:, ps_:ps_ + 128],
                                xt_t[e][:, q * 512:(q + 1) * 512],
                                start=(e == 0), stop=(e == NE - 1))
                        nc.vector.tensor_scalar_add(
                            kt_p[p][:, q * 512:(q + 1) * 512], pk[:, :],
                            bk_p[p][:, :])
                        pq = psA.tile([128, 512], f32, name="pq", tag="s")
                        for e in range(NE):
                            nc.tensor.matmul(
                                pq[:, :], wq_t[e][:, ps_:ps_ + 128],
                                xt_t[e][:, q * 512:(q + 1) * 512],
                                start=(e == 0), stop=(e == NE - 1))
                        nc.vector.tensor_scalar_add(
                            qt_p[p][:, q * 512:(q + 1) * 512], pq[:, :],
                            bq_p[p][:, :])

            # ---- Phase B: attention + output projection ----
                for qc in range(NQ if "B" in phases else 0):
                    q0 = qc * 512
                    nkt = (qc + 1) * (NLT // NQ)
                    pvh = [psV.tile([65, 512], f32, name="pvh", tag="pvh")
                           for _ in range(HPC)]
                    outT_p = [otp.tile([128, 512], f32r, name="otp", tag=f"otp{p}")
                              for p in range(NP)]
                    for kt in range(nkt):
                        # ea holds expA reversed along q so the DMA is
                        # contiguous; consumers read it with free step -1.
                        ea = [None] * n_g
                        earev = [None] * n_g
                        for r in range(n_g):
                            ea[r] = eap.tile([128, 512], f32, name=f"ea{r}",
                                             tag=f"ea{r}")
                            nc.gpsimd.dma_start(
                                ea[r][:, :],
                                bass.AP(g, r * GLEN + GOFF + kt * 128 - q0 - 511,
                                        [[1, 128], [1, 512]]))
                            base = ea[r][:, :]
                            pitch = base.ap[0][0]
                            earev[r] = bass.AP(ea[r].tensor, base.offset + 511,
                                               [[pitch, 128], [-1, 512]])
                        for pr in range(NP):
                            h0, h1 = 2 * pr, 2 * pr + 1
                            ps2 = psS.tile([128, 1024], f32, name="ps2", tag="s")
                            nc.tensor.matmul(
                                ps2[:, 0:512],
                                kt_p[pr][0:64, kt * 128:(kt + 1) * 128],
                                qt_p[pr][0:64, q0:q0 + 512],
                                start=True, stop=True, tile_position=(0, 0))
                            nc.tensor.matmul(
                                ps2[:, 512:1024],
                                kt_p[pr][64:128, kt * 128:(kt + 1) * 128],
                                qt_p[pr][64:128, q0:q0 + 512],
                                start=True, stop=True, tile_position=(64, 0))
                            # exp straight to the f32r tile, then multiply the
                            # decay factor in place (saves a staging tile)
                            p2 = wrk.tile([128, 1024], f32r, name="p2", bufs=5)
                            with nc.allow_low_precision(
                                    reason="exp output feeds f32r PV matmul"):
                                nc.scalar.activation(p2[:, :], ps2[:, :], Exp, scale=(0.0 if internal_io else 1.0))
                            nc.vector.tensor_mul(
                                p2[:, 0:512], p2[:, 0:512], earev[h0 % n_g])
                            nc.vector.tensor_mul(
                                p2[:, 512:1024], p2[:, 512:1024],
                                earev[h1 % n_g])
                            nc.tensor.matmul(
                                pvh[h0][:, :], vaug[kt][:, 65 * h0:65 * h0 + 65],
                                p2[:, 0:512],
                                start=(kt == 0), stop=(kt == nkt - 1))
                            nc.tensor.matmul(
                                pvh[h1][:, :], vaug[kt][:, 65 * h1:65 * h1 + 65],
                                p2[:, 512:1024],
                                start=(kt == 0), stop=(kt == nkt - 1))

                    for h in range(HPC):
                        pr, odd = h // 2, h % 2
                        rec = wrk.tile([1, 512], f32r, name="rec")
                        with nc.allow_low_precision(
                                reason="softmax denom reciprocal feeds PE broadcast"):
                            nc.vector.reciprocal(rec[:, :], pvh[h][64:65, :])
                        pbc = psS.tile([Hd, 512], f32, name="pbc", tag="s")
                        nc.tensor.matmul(pbc[:, :], ones64[:, :], rec[:, :],
                                         start=True, stop=True)
                        bc = wrk.tile([Hd, 512], f32, name="bc")
                        nc.vector.tensor_copy(bc[:, :], pbc[:, :])
                        if odd == 0:
                            nc.vector.tensor_mul(
                                outT_p[pr][0:64, :], pvh[h][0:64, :], bc[:, :])
                        else:
                            ostg = wrk.tile([Hd, 512], f32r, name="ostg", tag="fo")
                            nc.vector.tensor_mul(
                                ostg[:, :], pvh[h][0:64, :], bc[:, :])
                            nc.sync.dma_start(outT_p[pr][64:128, :], ostg[:, :])

                    for m in range(4 if "WO" in phases else 0):
                        for n in range(2):
                            pf = psV.tile([128, 512], f32, name="pf", tag="pvh")
                            for p in range(NP):
                                nc.tensor.matmul(
                                    pf[:, :],
                                    outT_p[p][:, m * 128:(m + 1) * 128],
                                    wo_p[p][:, n * 512:(n + 1) * 512],
                                    start=(p == 0), stop=(p == NP - 1))
                            fo = wrk.tile([128, 512], f32, name="fo")
                            nc.vector.tensor_copy(fo[:, :], pf[:, :])
                            nc.sync.dma_start(
                                out[q0 + m * 128:q0 + (m + 1) * 128,
                                    n * 512:(n + 1) * 512],
                                fo[:, :])
                            if internal_io and qc == NQ - 1 and m == 3 and n == 1:
                                nc.sync.dma_start(tok, fo[:, 0:1])

    _split_multi_waits(nc)
    return nc


_NC_CACHE = {}
_last_in_maps = None
_last_n_g = 1


def _get_nc(n_g):
    if n_g not in _NC_CACHE:
        _NC_CACHE[n_g] = build_nc(n_g)
    return _NC_CACHE[n_g]


def kernel(x, causal_mask, key_padding_mask, Wq, bq, Wk, bk, Wv, bv, Wo, bo,
           decay_logit):
    x = np.asarray(x, dtype=np.float32)
    Wq = np.asarray(Wq, dtype=np.float32)
    Wk = np.asarray(Wk, dtype=np.float32)
    Wv = np.asarray(Wv, dtype=np.float32)
    Wo = np.asarray(Wo, dtype=np.float32)
    bq = np.asarray(bq, dtype=np.float32)
    bk = np.asarray(bk, dtype=np.float32)
    bv = np.asarray(bv, dtype=np.float32)
    bo = np.asarray(bo, dtype=np.float32)
    decay_logit = np.asarray(decay_logit, dtype=np.float32)
    key_padding_mask = np.asarray(key_padding_mask)

    scale = 1.0 / math.sqrt(Hd)
    rates = np.log1p(np.exp(decay_logit.astype(np.float64)))  # softplus [H]

    def g_vec(rate):
        d = np.arange(GLEN) - GOFF           # d = q - k in [-2047, 2047]
        vals = np.where(d >= 0, (1.0 + np.abs(d)) ** (-rate), 0.0)
        # device AP reads g[GOFF + k - q] => store reversed
        return vals[::-1].astype(np.float32)

    in_maps = []
    n_g_needed = 1
    for c in range(N_CORES):
        b = c // CPB
        hs = (c % CPB) * HPC                 # first head of this core
        sl = slice(hs * Hd, (hs + HPC) * Hd)
        core_rates = rates[hs:hs + HPC]
        if not np.allclose(core_rates, core_rates[0], rtol=1e-6, atol=1e-9):
            n_g_needed = HPC
        gmat = (np.stack([g_vec(core_rates[0])])
                if n_g_needed == 1
                else np.stack([g_vec(r) for r in core_rates]))
        in_maps.append({
            "xT": np.ascontiguousarray(x[b].T),
            "wqT": np.ascontiguousarray((Wq[sl] * scale).T),
            "wkT": np.ascontiguousarray(Wk[sl].T),
            "wvT": np.ascontiguousarray(Wv[sl].T),
            "woT": np.ascontiguousarray(Wo[:, sl].T),
            "bq": np.ascontiguousarray((bq[sl] * scale).reshape(DHC, 1)),
            "bk": np.ascontiguousarray(bk[sl].reshape(DHC, 1)),
            "g": gmat,
            "pmask": np.ascontiguousarray(
                (~key_padding_mask[b]).astype(np.float32).reshape(L, 1)),
        })

    global _last_in_maps, _last_n_g
    _last_in_maps, _last_n_g = in_maps, n_g_needed
    nc = _get_nc(n_g_needed)
    res = bass_utils.run_bass_kernel_spmd(
        nc, in_maps, core_ids=list(range(N_CORES)))

    # q-independent constant: Wo @ bv + bo (softmax rows sum to 1)
    const = Wo.astype(np.float64) @ bv.astype(np.float64) + bo
    out = np.zeros((B, L, D), dtype=np.float64)
    for c in range(N_CORES):
        out[c // CPB] += res.results[c]["out"]
    out += const[None, None, :]
    return out.astype(np.float32)


# revision 24
# speedup vs baseline: 1.1282x; 1.1282x over previous
"""DecayAttention Trainium2 kernel — 8-core SPMD.

Problem: B=2, L=2048, D=1024, H=16 heads (Hd=64).
  out = (softmax(Q K^T/sqrt(Hd) - rate_h*log1p(|i-j|) + causal) V) @ Wo.T + bo

Sharding: core c handles batch b = c//4 and heads h in [4*(c%4), 4*(c%4)+4).
Q/K/V projections column-sharded, Wo row-sharded; the 4 cores of each batch
return partial outputs that the host sums (plus Wo@bv + bo, both of which are
q-independent constants because softmax rows sum to 1).

Device-side layout tricks:
  - x is shipped pre-transposed (xT [D, L]) so every matmul contraction dim
    sits on partitions; no on-device transposes at all.
  - Q^T/K^T [64, L] per head come straight out of the projection matmuls.
  - scores are computed transposed (S^T[k, q] = K Q^T) so softmax's k-reduction
    becomes a matmul contraction: V is augmented with a ones column and
    P^T = exp(S^T) * expA gives numerator and denominator in one PV matmul.
  - decay bias + causal mask collapse into one Toeplitz factor
    expA[k, q] = (1+|q-k|)^(-rate) * [k <= q], materialized per tile by a
    single DMA from a 4095-float vector with a diagonal access pattern
    [[1, 128], [-1, 512]].
  - matmul operands use float32r (TF32, full PE rate at N>=256).
"""
import math

import numpy as np

import concourse.bass as bass
import concourse.mybir as mybir
import concourse.tile as tile
from concourse import bass_utils

f32 = mybir.dt.float32
f32r = mybir.dt.float32r
Exp = mybir.ActivationFunctionType.Exp

B, L, D, H = 2, 2048, 1024, 16
Hd = D // H                      # 64
N_CORES = 8
CPB = N_CORES // B               # 4 cores per batch element
HPC = H // CPB                   # 4 heads per core
DHC = HPC * Hd                   # 256 head-dims per core
NQ = L // 512                    # 4 q-chunks of 512
NLT = L // 128                   # 16 l/k tiles of 128
NE = D // 128                    # 8 contraction tiles for projections
GLEN = 2 * L - 1                 # 4095
GOFF = L - 1                     # 2047


def _split_multi_waits(nc):
    """This container's walrus accepts at most one sync-wait per engine
    instruction; hoist extras onto single-wait NOPs placed just before."""
    for fn in nc.m.functions:
        for blk in fn.blocks:
            out, changed = [], False
            for inst in blk.instructions:
                si = inst.sync_info
                waits = list(si.on_wait) if si is not None and si.on_wait else []
                if len(waits) > 1:
                    changed = True
                    for w in waits[:-1]:
                        nop = mybir.InstNoOp(
                            name=nc.get_next_instruction_name(), ins=[], outs=[])
                        nop.engine = inst.engine
                        nop.sync_info = mybir.SyncInfo(on_wait=[w], on_update=[])
                        out.append(nop)
                    inst.sync_info = mybir.SyncInfo(
                        on_wait=[waits[-1]], on_update=list(si.on_update or []))
                out.append(inst)
            if changed:
                blk.instructions = out


def build_nc(n_g: int, phases=("A", "B", "WO"), repeat=1, internal_io=False):
    """Build the per-core Bass program. n_g = 1 (all heads share one decay
    rate, the setup_inputs case) or HPC (per-head expA vectors)."""
    nc = bass.Bass("TRN2", target_bir_lowering=False, debug=False)

    big = "Internal" if internal_io else "ExternalInput"
    xT = nc.dram_tensor("xT", [D, L], f32r, kind=big).ap()
    wqT = nc.dram_tensor("wqT", [D, DHC], f32r, kind=big).ap()
    wkT = nc.dram_tensor("wkT", [D, DHC], f32r, kind=big).ap()
    wvT = nc.dram_tensor("wvT", [D, DHC], f32r, kind=big).ap()
    woT = nc.dram_tensor("woT", [DHC, D], f32r, kind=big).ap()
    bq = nc.dram_tensor("bq", [DHC, 1], f32, kind="ExternalInput").ap()
    bk = nc.dram_tensor("bk", [DHC, 1], f32, kind="ExternalInput").ap()
    g = nc.dram_tensor("g", [n_g, GLEN], f32, kind="ExternalInput")
    pmask = nc.dram_tensor("pmask", [L, 1], f32, kind="ExternalInput").ap()
    out = nc.dram_tensor(
        "out", [L, D], f32,
        kind="Internal" if internal_io else "ExternalOutput").ap()
    tok = (nc.dram_tensor("tok", [128, 1], f32, kind="ExternalOutput").ap()
           if internal_io else None)

    with tile.TileContext(nc) as tc:
      for _rep in range(repeat):
        with tc.tile_pool(name="cons", bufs=1) as cons:
            # persistent SBUF residents (pair layout: pair p = heads 2p, 2p+1)
            NP = HPC // 2
            qt_p = [cons.tile([128, L], f32r, name=f"qt{p}") for p in range(NP)]
            kt_p = [cons.tile([128, L], f32r, name=f"kt{p}") for p in range(NP)]
            vaug = [cons.tile([128, 65 * HPC], f32r, name=f"vaug{t}")
                    for t in range(NLT)]
            wo_p = [cons.tile([128, D], f32r, name=f"wo{p}") for p in range(NP)]
            bq_p = [cons.tile([128, 1], f32, name=f"bq{p}") for p in range(NP)]
            bk_p = [cons.tile([128, 1], f32, name=f"bk{p}") for p in range(NP)]
            ones64 = cons.tile([1, Hd], f32r)
            ones_st = cons.tile([128, HPC], f32)

            ones_st64 = cons.tile([1, Hd], f32)
            nc.vector.memset(ones_st[:, :], 1.0)
            nc.vector.memset(ones_st64[:, :], 1.0)
            nc.vector.tensor_copy(ones64[:, :], ones_st64[:, :])
            for p in range(NP):
                nc.sync.dma_start(wo_p[p][:, :], woT[p * 128:(p + 1) * 128, :])
                nc.sync.dma_start(bq_p[p][:, :], bq[p * 128:(p + 1) * 128, :])
                nc.sync.dma_start(bk_p[p][:, :], bk[p * 128:(p + 1) * 128, :])

            # ---- Phase A + B pools (single PSUM pool layout: tag "s"
            # slots are [128,1024]=2 banks x2 bufs; phase-A tiles borrow them)
            with tc.tile_pool(name="eap", bufs=5) as eap, \
                 tc.tile_pool(name="wrk", bufs=3) as wrk, \
                 tc.tile_pool(name="otp", bufs=2) as otp, \
                 tc.tile_pool(name="psS", bufs=2, space="PSUM") as psS, \
                 tc.tile_pool(name="psV", bufs=HPC, space="PSUM") as psV, \
                 tc.tile_pool(name="xw", bufs=1) as xw:
                psA = psS
                xt_t = [xw.tile([128, L], f32r, name=f"x{e}") for e in range(NE)]
                wq_t = [xw.tile([128, DHC], f32r, name=f"wq{e}") for e in range(NE)]
                wk_t = [xw.tile([128, DHC], f32r, name=f"wk{e}") for e in range(NE)]
                wv_t = [xw.tile([128, DHC], f32r, name=f"wv{e}") for e in range(NE)]
                pm_t = [xw.tile([128, 1], f32, name=f"pm{t}") for t in range(NLT)]
                for e in range(NE):
                    nc.sync.dma_start(xt_t[e][:, :], xT[e * 128:(e + 1) * 128, :])
                    nc.sync.dma_start(wq_t[e][:, :], wqT[e * 128:(e + 1) * 128, :])
                    nc.sync.dma_start(wk_t[e][:, :], wkT[e * 128:(e + 1) * 128, :])
                    nc.sync.dma_start(wv_t[e][:, :], wvT[e * 128:(e + 1) * 128, :])
                for t in range(NLT):
                    nc.sync.dma_start(pm_t[t][:, :], pmask[t * 128:(t + 1) * 128, :])

                # Projections grouped by q-chunk: V l-tiles 4qc..4qc+3,
                # then K and Q for that chunk — attention(qc) unblocks early.
                def v_tile(t):
                    pv = psA.tile([128, DHC], f32, name="pv", tag="s")
                    for e in range(NE):
                        nc.tensor.matmul(
                            pv[:, :], xt_t[e][:, t * 128:(t + 1) * 128],
                            wv_t[e][:, :],
                            start=(e == 0), stop=(e == NE - 1))
                    dst = bass.AP(vaug[t].tensor, 0,
                                  [[65 * HPC, 128], [65, HPC], [1, Hd]])
                    src_ = bass.AP(pv.tensor, 0,
                                   [[DHC, 128], [Hd, HPC], [1, Hd]])
                    nc.vector.tensor_scalar_mul(dst, src_, pm_t[t][:, :])
                    ones_dst = bass.AP(vaug[t].tensor, Hd,
                                       [[65 * HPC, 128], [65, HPC]])
                    nc.vector.tensor_scalar_mul(ones_dst, ones_st[:, :],
                                                pm_t[t][:, :])

                for q in range(NQ):
                    for t in range(4 * q, 4 * q + 4):
                        v_tile(t)
                    for p in range(NP):
                        ps_ = p * 128
                        pk = psA.tile([128, 512], f32, name="pk", tag="s")
                        for e in range(NE):
                            nc.tensor.matmul(
                                pk[:, :], wk_t[e][:, ps_:ps_ + 128],
                                xt_t[e][:, q * 512:(q + 1) * 512],
                                start=(e == 0), stop=(e == NE - 1))
                        nc.vector.tensor_scalar_add(
                            kt_p[p][:, q * 512:(q + 1) * 512], pk[:, :],
                            bk_p[p][:, :])
                        pq = psA.tile([128, 512], f32, name="pq", tag="s")
                        for e in range(NE):
                            nc.tensor.matmul(
                                pq[:, :], wq_t[e][:, ps_:ps_ + 128],
                                xt_t[e][:, q * 512:(q + 1) * 512],
                                start=(e == 0), stop=(e == NE - 1))
                        nc.vector.tensor_scalar_add(
                            qt_p[p][:, q * 512:(q + 1) * 512], pq[:, :],
                            bq_p[p][:, :])

            # ---- Phase B: attention + output projection ----
                for qc in range(NQ if "B" in phases else 0):
                    q0 = qc * 512
                    nkt = (qc + 1) * (NLT // NQ)
                    pvh = [psV.tile([65, 512], f32, name="pvh", tag="pvh")
                           for _ in range(HPC)]
                    outT_p = [otp.tile([128, 512], f32r, name="otp", tag=f"otp{p}")
                              for p in range(NP)]
                    for kt in range(nkt):
                        # ea holds expA reversed along q so the DMA is
                        # contiguous; consumers read it with free step -1.
                        ea = [None] * n_g
                        earev = [None] * n_g
                        for r in range(n_g):
                            ea[r] = eap.tile([128, 512], f32, name=f"ea{r}",
                                             tag=f"ea{r}")
                            nc.gpsimd.dma_start(
                                ea[r][:, :],
                                bass.AP(g, r * GLEN + GOFF + kt * 128 - q0 - 511,
                                        [[1, 128], [1, 512]]))
                            base = ea[r][:, :]
                            pitch = base.ap[0][0]
                            earev[r] = bass.AP(ea[r].tensor, base.offset + 511,
                                               [[pitch, 128], [-1, 512]])
                        for pr in range(NP):
                            h0, h1 = 2 * pr, 2 * pr + 1
                            ps2 = psS.tile([128, 1024], f32, name="ps2", tag="s")
                            nc.tensor.matmul(
                                ps2[:, 0:512],
                                kt_p[pr][0:64, kt * 128:(kt + 1) * 128],
                                qt_p[pr][0:64, q0:q0 + 512],
                                start=True, stop=True, tile_position=(0, 0))
                            nc.tensor.matmul(
                                ps2[:, 512:1024],
                                kt_p[pr][64:128, kt * 128:(kt + 1) * 128],
                                qt_p[pr][64:128, q0:q0 + 512],
                                start=True, stop=True, tile_position=(64, 0))
                            # exp straight to the f32r tile, then multiply the
                            # decay factor in place (saves a staging tile)
                            p2 = wrk.tile([128, 1024], f32r, name="p2", bufs=5)
                            with nc.allow_low_precision(
                                    reason="exp output feeds f32r PV matmul"):
                                nc.scalar.activation(p2[:, :], ps2[:, :], Exp, scale=(0.0 if internal_io else 1.0))
                            nc.vector.tensor_mul(
                                p2[:, 0:512], p2[:, 0:512], earev[h0 % n_g])
                            nc.vector.tensor_mul(
                                p2[:, 512:1024], p2[:, 512:1024],
                                earev[h1 % n_g])
                            nc.tensor.matmul(
                                pvh[h0][:, :], vaug[kt][:, 65 * h0:65 * h0 + 65],
                                p2[:, 0:512],
                                start=(kt == 0), stop=(kt == nkt - 1))
                            nc.tensor.matmul(
                                pvh[h1][:, :], vaug[kt][:, 65 * h1:65 * h1 + 65],
                                p2[:, 512:1024],
                                start=(kt == 0), stop=(kt == nkt - 1))

                    for h in range(HPC):
                        pr, odd = h // 2, h % 2
                        rec = wrk.tile([1, 512], f32r, name="rec")
                        with nc.allow_low_precision(
                                reason="softmax denom reciprocal feeds PE broadcast"):
                            nc.vector.reciprocal(rec[:, :], pvh[h][64:65, :])
                        pbc = psS.tile([Hd, 512], f32, name="pbc", tag="s")
                        nc.tensor.matmul(pbc[:, :], ones64[:, :], rec[:, :],
                                         start=True, stop=True)
                        bc = wrk.tile([Hd, 512], f32, name="bc")
                        nc.vector.tensor_copy(bc[:, :], pbc[:, :])
                        if odd == 0:
                            nc.vector.tensor_mul(
                                outT_p[pr][0:64, :], pvh[h][0:64, :], bc[:, :])
                        else:
                            ostg = wrk.tile([Hd, 512], f32r, name="ostg", tag="fo")
                            nc.vector.tensor_mul(
                                ostg[:, :], pvh[h][0:64, :], bc[:, :])
                            nc.sync.dma_start(outT_p[pr][64:128, :], ostg[:, :])

                    for m in range(4 if "WO" in phases else 0):
                        for n in range(2):
                            pf = psV.tile([128, 512], f32, name="pf", tag="pvh")
                            for p in range(NP):
                                nc.tensor.matmul(
                                    pf[:, :],
                                    outT_p[p][:, m * 128:(m + 1) * 128],
                                    wo_p[p][:, n * 512:(n + 1) * 512],
                                    start=(p == 0), stop=(p == NP - 1))
                            fo = wrk.tile([128, 512], f32, name="fo")
                            nc.vector.tensor_copy(fo[:, :], pf[:, :])
                            nc.sync.dma_start(
                                out[q0 + m * 128:q0 + (m + 1) * 128,
                                    n * 512:(n + 1) * 512],
                                fo[:, :])
                            if internal_io and qc == NQ - 1 and m == 3 and n == 1:
                                nc.sync.dma_start(tok, fo[:, 0:1])

    _split_multi_waits(nc)
    return nc


_NC_CACHE = {}
_last_in_maps = None
_last_n_g = 1


def _get_nc(n_g):
    if n_g not in _NC_CACHE:
        _NC_CACHE[n_g] = build_nc(n_g)
    return _NC_CACHE[n_g]


def kernel(x, causal_mask, key_padding_mask, Wq, bq, Wk, bk, Wv, bv, Wo, bo,
           decay_logit):
    x = np.asarray(x, dtype=np.float32)
    Wq = np.asarray(Wq, dtype=np.float32)
    Wk = np.asarray(Wk, dtype=np.float32)
    Wv = np.asarray(Wv, dtype=np.float32)
    Wo = np.asarray(Wo, dtype=np.float32)
    bq = np.asarray(bq, dtype=np.float32)
    bk = np.asarray(bk, dtype=np.float32)
    bv = np.asarray(bv, dtype=np.float32)
    bo = np.asarray(bo, dtype=np.float32)
    decay_logit = np.asarray(decay_logit, dtype=np.float32)
    key_padding_mask = np.asarray(key_padding_mask)

    scale = 1.0 / math.sqrt(Hd)
    rates = np.log1p(np.exp(decay_logit.astype(np.float64)))  # softplus [H]

    def g_vec(rate):
        d = np.arange(GLEN) - GOFF           # d = q - k in [-2047, 2047]
        vals = np.where(d >= 0, (1.0 + np.abs(d)) ** (-rate), 0.0)
        # device AP reads g[GOFF + k - q] => store reversed
        return vals[::-1].astype(np.float32)

    in_maps = []
    n_g_needed = 1
    for c in range(N_CORES):
        b = c // CPB
        hs = (c % CPB) * HPC                 # first head of this core
        sl = slice(hs * Hd, (hs + HPC) * Hd)
        core_rates = rates[hs:hs + HPC]
        if not np.allclose(core_rates, core_rates[0], rtol=1e-6, atol=1e-9):
            n_g_needed = HPC
        gmat = (np.stack([g_vec(core_rates[0])])
                if n_g_needed == 1
                else np.stack([g_vec(r) for r in core_rates]))
        in_maps.append({
            "xT": np.ascontiguousarray(x[b].T),
            "wqT": np.ascontiguousarray((Wq[sl] * scale).T),
            "wkT": np.ascontiguousarray(Wk[sl].T),
            "wvT": np.ascontiguousarray(Wv[sl].T),
            "woT": np.ascontiguousarray(Wo[:, sl].T),
            "bq": np.ascontiguousarray((bq[sl] * scale).reshape(DHC, 1)),
            "bk": np.ascontiguousarray(bk[sl].reshape(DHC, 1)),
            "g": gmat,
            "pmask": np.ascontiguousarray(
                (~key_padding_mask[b]).astype(np.float32).reshape(L, 1)),
        })

    global _last_in_maps, _last_n_g
    _last_in_maps, _last_n_g = in_maps, n_g_needed
    nc = _get_nc(n_g_needed)
    res = bass_utils.run_bass_kernel_spmd(
        nc, in_maps, core_ids=list(range(N_CORES)))

    # q-independent constant: Wo @ bv + bo (softmax rows sum to 1)
    const = Wo.astype(np.float64) @ bv.astype(np.float64) + bo
    out = np.zeros((B, L, D), dtype=np.float64)
    for c in range(N_CORES):
        out[c // CPB] += res.results[c]["out"]
    out += const[None, None, :]
    return out.astype(np.float32)
